# revision 34
# baseline (speedup 1.0000x reference)
"""Trainium2 Bass kernel for nn_AdaptedEditFlowsTransformer.

Self-contained: takes full (unsharded) inputs, returns the full output tuple
(rates, ins, sub) matching the reference.

Sharding over 8 NeuronCores:
  - transformer trunk: sequence-parallel, 256 tokens per core (4 cores per
    batch element); K/V all-gathered per layer within each batch's core group.
  - output heads (ins/sub, V=32000): vocab-parallel, 4000 columns per core;
    softmax denominators combined with one small AllReduce per head.
  - rates head: token-parallel (each core emits its 256 tokens).

Host-side prep is limited to layout/dtype work: embedding row gather, bf16
weight casts, folding LayerNorm gains/biases + 1/sqrt(dh) into adjacent
matmul weights, and the time-embedding contribution folded into head biases.
"""

import sys

sys.path.insert(0, "/opt/trn_rl_repo")

import math

import numpy as np
import ml_dtypes

BF16 = ml_dtypes.bfloat16

B, L, D, H, NL = 2, 1024, 1024, 16, 2
V, TD, FF = 32000, 512, 4096
DH = D // H  # 64
N_CORES = 8
T = (B * L) // N_CORES  # 256 tokens per core
TJ = T // 128  # 2 token sub-tiles per core
VS = V // N_CORES  # 4000 vocab cols per core
NVT = 8  # vocab n-tiles per core
VT = VS // NVT  # 500
MASK_NEG = -60.0
EPS = 1e-5

_cache = {}


def _build(general_bias):
    import concourse.bass as bass
    import concourse.tile as tile
    import concourse.mybir as mybir
    from concourse import bacc
    from concourse.masks import make_identity

    f32 = mybir.dt.float32
    bf16 = mybir.dt.bfloat16
    AF = mybir.ActivationFunctionType
    Alu = mybir.AluOpType

    nc = bacc.Bacc("TRN2", target_bir_lowering=False, debug=False,
                   num_devices=N_CORES)

    # ---------------- DRAM parameters ----------------
    def din(name, shape, dt=f32):
        return nc.dram_tensor(name, shape, dt, kind="ExternalInput")

    x0_d = din("x0", [T, D])                     # embed[tokens]+pos, own tokens
    maskT_d = din("maskT", [L, T], bf16)         # attn 0/1 multiplier, [k, own q]
    pad_all_d = din("pad_all", [B * L])          # (~pad) as f32, all tokens
    pad_own_d = din("pad_own", [T])              # (~pad) f32, own tokens
    wqkv_d, bq_d, bkv_d, wo_d, bo_d = [], [], [], [], []
    wup_d, bup_d, wdn_d, bdn_d = [], [], [], []
    for i in range(NL):
        wqkv_d.append(din(f"wqkv{i}", [D, 3 * D], bf16))
        bq_d.append(din(f"bq{i}", [D]))
        bkv_d.append(din(f"bkv{i}", [2 * D]))
        wo_d.append(din(f"wo{i}", [DH, H, D], bf16))   # host pre-shuffled
        bo_d.append(din(f"bo{i}", [D]))
        wup_d.append(din(f"wup{i}", [D, FF], bf16))
        bup_d.append(din(f"bup{i}", [FF]))
        wdn_d.append(din(f"wdn{i}", [FF, D], bf16))
        bdn_d.append(din(f"bdn{i}", [D]))
    w1r_d = din("w1r", [D, TD], bf16)
    b1r_d = din("b1r", [TD])
    w2r_d = din("w2r", [TD, 3], bf16)
    b2r_d = din("b2r", [1, 3])
    w1i_d = din("w1i", [D, TD], bf16)
    b1i_d = din("b1i", [TD])
    w2i_d = din("w2i", [TD, VS], bf16)
    b2i_d = din("b2i", [1, VS], bf16)
    w1s_d = din("w1s", [D, TD], bf16)
    b1s_d = din("b1s", [TD])
    w2s_d = din("w2s", [TD, VS], bf16)
    b2s_d = din("b2s", [1, VS], bf16)

    out_ins_d = nc.dram_tensor("out_ins", [B * L, VS], f32, kind="ExternalOutput")
    out_sub_d = nc.dram_tensor("out_sub", [B * L, VS], f32, kind="ExternalOutput")
    out_rates_d = nc.dram_tensor("out_rates", [T, 3], f32, kind="ExternalOutput")

    # collective bounce buffers
    kv_in = [nc.dram_tensor(f"kv_in{i}", [2 * D * T], bf16) for i in range(NL)]
    kv_all = [nc.dram_tensor(f"kv_all{i}", [4 * 2 * D * T], bf16)
              for i in range(NL)]
    hid_in = nc.dram_tensor("hid_in", [2 * TD, T], bf16)
    hid_all = nc.dram_tensor("hid_all", [N_CORES * 2 * TD, T], bf16,
                             addr_space="Shared")
    ar_in = [nc.dram_tensor(f"ar_in{k}", [B * L // 4], f32) for k in range(8)]
    ssum_d = nc.dram_tensor("ssum_d", [H, T], f32)
    ar_out = [nc.dram_tensor(f"ar_out{k}", [B * L // 4], f32, addr_space="Shared")
              for k in range(8)]

    groups_batch = [[0, 1, 2, 3], [4, 5, 6, 7]]
    groups_all = [list(range(N_CORES))]

    with tile.TileContext(nc) as tc:
        with (
            tc.tile_pool(name="persist", bufs=1) as pp,
            tc.tile_pool(name="ps1", bufs=4, space="PSUM") as ps1,
            tc.tile_pool(name="ps2", bufs=2, space="PSUM") as ps2,
        ):
            ident = pp.tile([128, 128], f32, tag="ident")
            make_identity(nc, ident[:])
            ident_bf = pp.tile([64, 64], bf16, tag="ident_bf")
            make_identity(nc, ident_bf[:])
            ones65 = pp.tile([65, 128], f32, tag="ones65")
            nc.vector.memset(ones65[:], 1.0)
            eps_t = pp.tile([128, 1], f32, tag="eps")
            nc.vector.memset(eps_t[:], EPS)

            # resident activations
            x_sb = pp.tile([128, TJ, D], f32, tag="x")         # residual
            x0_v = x0_d.ap().rearrange("(j p) d -> p j d", p=128)
            for j in range(TJ):
                nc.sync.dma_start(x_sb[:, j:j + 1, :], x0_v[:, j:j + 1, :])
            xnT = pp.tile([128, D // 128, T], bf16, tag="xnT")  # normed, transposed

            def layer_norm_normalize(sp):
                """x_sb -> xnT (pure (x-m)*rstd, transposed, bf16)."""
                for j in range(TJ):
                    xj = x_sb[:, j, :]
                    stats = sp.tile([128, D // 512, 6], f32, tag="ln_stats")
                    for g in range(D // 512):
                        nc.vector.bn_stats(stats[:, g, :], xj[:, g * 512:(g + 1) * 512])
                    mv = sp.tile([128, 2], f32, tag="ln_mv")
                    nc.vector.bn_aggr(mv[:], stats[:])
                    rstd = sp.tile([128, 1], f32, tag="ln_rstd")
                    nc.scalar.activation(rstd[:], mv[:, 1:2], AF.Sqrt, bias=eps_t[:])
                    nc.vector.reciprocal(rstd[:], rstd[:])
                    nmr = sp.tile([128, 1], f32, tag="ln_nmr")
                    nc.vector.tensor_scalar(nmr[:], mv[:, 0:1], rstd[:], -1.0,
                                            Alu.mult, Alu.mult)
                    xn = sp.tile([128, D], f32, tag="ln_xn")
                    nc.vector.tensor_scalar(xn[:], xj, rstd[:], nmr[:],
                                            Alu.mult, Alu.add)
                    for dt_ in range(D // 128):
                        pt = ps2.tile([128, 128], f32, tag="tp")
                        nc.tensor.transpose(pt[:], xn[:, dt_ * 128:(dt_ + 1) * 128],
                                            ident[:])
                        nc.vector.tensor_copy(xnT[:, dt_, j * 128:(j + 1) * 128], pt[:])

            # ============================ trunk ============================
            with (
                tc.tile_pool(name="t_w", bufs=3) as wp,
                tc.tile_pool(name="t_acts", bufs=1) as ap_,
                tc.tile_pool(name="t_sp", bufs=3) as sp,
            ):
                maskT_sb = ap_.tile([128, L // 128, T], bf16, tag="maskT")
                nc.sync.dma_start(
                    maskT_sb[:], maskT_d.ap().rearrange("(kt p) q -> p kt q", p=128))
                for li in range(NL):
                    layer_norm_normalize(sp)

                    # ---- qkv projection ----
                    wqkv_v = wqkv_d[li].ap().rearrange("(kt p) m -> p kt m", p=128)
                    wqkv_h = []
                    for hv in range(2):
                        wt = wp.tile([128, D // 128, 3 * D // 2], bf16, tag="ws")
                        for kk in range(0, D // 128, 2):
                            nc.sync.dma_start(
                                wt[:, kk:kk + 2, :],
                                wqkv_v[:, kk:kk + 2,
                                       hv * (3 * D // 2):(hv + 1) * (3 * D // 2)])
                        wqkv_h.append(wt)
                    def wqkv_sl(k, lo, hi):
                        half = lo // (3 * D // 2)
                        off = lo - half * (3 * D // 2)
                        return wqkv_h[half][:, k, off:off + (hi - lo)]
                    bq_sb = sp.tile([128, 8], f32, tag="bq")
                    nc.sync.dma_start(bq_sb[:], bq_d[li].ap().rearrange("(m p) -> p m", p=128))
                    bk_sb = sp.tile([128, 8], f32, tag="bkv")
                    nc.sync.dma_start(bk_sb[:],
                                      bkv_d[li].ap()[0:D].rearrange("(m p) -> p m", p=128))
                    bv_bc = ap_.tile([128, D], f32, tag="bv_bc")
                    nc.sync.dma_start(
                        bv_bc[:],
                        bass.AP(tensor=bkv_d[li], offset=D, ap=[[0, 128], [1, D]]))

                    kvT = ap_.tile([128, 8, T], bf16, tag="kvoT")   # K, [dim, tok]
                    v_nat_own = ap_.tile([128, TJ, D], bf16, tag="vno")
                    for m in range(8):  # K tiles first so the all-gather launches early
                        pkv = ps1.tile([128, T], f32, tag="pW")
                        for k in range(D // 128):
                            nc.tensor.matmul(pkv[:], wqkv_sl(k, m * 128, (m + 1) * 128),
                                             xnT[:, k, :], start=(k == 0),
                                             stop=(k == D // 128 - 1))
                        nc.vector.tensor_scalar_add(kvT[:, m, :], pkv[:], bk_sb[:, m:m + 1])
                        nc.sync.dma_start(
                            bass.AP(tensor=kv_in[li], offset=m * 128 * T,
                                    ap=[[T, 128], [1, T]]),
                            kvT[:, m, :])
                    for j in range(TJ):  # V directly in natural [token, dim] layout
                        for n in range(2):
                            pv = ps1.tile([128, 512], f32, tag="pW")
                            for k in range(D // 128):
                                nc.tensor.matmul(
                                    pv[:], xnT[:, k, j * 128:(j + 1) * 128],
                                    wqkv_sl(k, D + n * 512, D + (n + 1) * 512),
                                    start=(k == 0), stop=(k == D // 128 - 1))
                            nc.vector.tensor_add(v_nat_own[:, j, n * 512:(n + 1) * 512],
                                                 pv[:], bv_bc[:, n * 512:(n + 1) * 512])
                            nc.sync.dma_start(
                                bass.AP(tensor=kv_in[li],
                                        offset=D * T + j * 128 * D + n * 512,
                                        ap=[[D, 128], [1, 512]]),
                                v_nat_own[:, j, n * 512:(n + 1) * 512])
                    nc.gpsimd.collective_compute(
                        "AllGather", Alu.bypass, replica_groups=groups_batch,
                        ins=[kv_in[li].ap().opt()], outs=[kv_all[li].ap().opt()])

                    qtmp = ap_.tile([128, 8, T], bf16, tag="qT")  # overlaps the AG
                    for m in range(8):
                        pq = ps1.tile([128, T], f32, tag="pW")
                        for k in range(D // 128):
                            nc.tensor.matmul(pq[:], wqkv_sl(k, 2 * D + m * 128, 2 * D + (m + 1) * 128),
                                             xnT[:, k, :], start=(k == 0),
                                             stop=(k == D // 128 - 1))
                        nc.vector.tensor_scalar_add(qtmp[:, m, :], pq[:], bq_sb[:, m:m + 1])

                    # ---- attention ----
                    wo_sb = wp.tile([64, H, D], bf16, tag="ws")
                    for kk in range(0, H, 8):
                        nc.sync.dma_start(wo_sb[:, kk:kk + 8, :],
                                          wo_d[li].ap()[:, kk:kk + 8, :])
                    oT = ap_.tile([64, H, T], bf16, tag="oT")
                    CH = 2 * D * T  # flat chunk stride in kv_all
                    for h in [hh for par in range(2) for hh in range(par, H, 2)]:
                        po = (h % 2) * 64
                        kT2 = sp.tile([128, 4, T], bf16, tag="kT")
                        nc.sync.dma_start(
                            kT2[po:po + 64, :, :],
                            bass.AP(tensor=kv_all[li],
                                    offset=(h // 2) * 128 * T + po * T,
                                    ap=[[T, 64], [CH, 4], [1, T]]))
                        # v for this head + ones row (softmax denominators)
                        v_aug = sp.tile([128, L // 128, DH + 1], bf16, tag="vnat")
                        nc.vector.memset(v_aug[:, :, DH:DH + 1], 1.0)
                        _va = v_aug[:]
                        for jj in range(TJ):
                            nc.sync.dma_start(
                                bass.AP(tensor=_va.tensor,
                                        offset=_va.offset + jj * (DH + 1),
                                        ap=[[8 * (DH + 1), 128],
                                            [2 * (DH + 1), 4], [1, DH]]),
                                bass.AP(tensor=kv_all[li],
                                        offset=D * T + h * DH + jj * 128 * D,
                                        ap=[[D, 128], [CH, 4], [1, DH]]))
                        expT = sp.tile([128, L // 128, T], bf16, tag="expT")
                        for kt in range(L // 128):
                            ps = ps1.tile([128, T], f32, tag="pW")
                            nc.tensor.matmul(
                                ps[:],
                                kT2[po:po + 64, kt // 2, (kt % 2) * 128:(kt % 2) * 128 + 128],
                                qtmp[po:po + 64, h // 2, :], start=True, stop=True)
                            er = sp.tile([128, T], bf16, tag="expr")
                            nc.scalar.activation(er[:], ps[:], AF.Exp)
                            nc.gpsimd.tensor_mul(expT[:, kt, :], er[:], maskT_sb[:, kt, :])
                        pav = ps2.tile([65, T], f32, tag="pav")
                        for kt in range(L // 128):
                            nc.tensor.matmul(pav[:], v_aug[:, kt, :], expT[:, kt, :],
                                             start=(kt == 0), stop=(kt == L // 128 - 1))
                        ssum = sp.tile([65, T], f32, tag="recip")
                        nc.vector.tensor_copy(ssum[64:65, :], pav[64:65, :])
                        nc.sync.dma_start(ssum_d.ap()[h, :], ssum[64:65, :])
                        rbs = sp.tile([64, T], f32, tag="rbs")
                        nc.sync.dma_start(
                            rbs[:], bass.AP(tensor=ssum_d, offset=h * T,
                                            ap=[[0, 64], [1, T]]))
                        rb = sp.tile([64, T], f32, tag="rb")
                        nc.vector.reciprocal(rb[:], rbs[:])
                        nc.vector.tensor_mul(oT[:, h, :], pav[0:64, :], rb[:])

                    # ---- o @ wo + bo, residual add ----
                    bo_bc = ap_.tile([128, D], f32, tag="bo_bc")
                    nc.sync.dma_start(
                        bo_bc[:],
                        bass.AP(tensor=bo_d[li], offset=0, ap=[[0, 128], [1, D]]))
                    for j in range(TJ):
                        for n in range(2):
                            pw = ps1.tile([128, 512], f32, tag="pW")
                            for h in range(H):
                                nc.tensor.matmul(pw[:], oT[:, h, j * 128:(j + 1) * 128],
                                                 wo_sb[:, h, n * 512:(n + 1) * 512],
                                                 start=(h == 0), stop=(h == H - 1))
                            xs = x_sb[:, j, n * 512:(n + 1) * 512]
                            nc.vector.tensor_add(xs, xs, pw[:])
                            nc.vector.tensor_add(xs, xs, bo_bc[:, n * 512:(n + 1) * 512])

                    # ---- MLP ----
                    layer_norm_normalize(sp)
                    wup_v = wup_d[li].ap().rearrange("(kt p) m -> p kt m", p=128)
                    wup_h = []
                    for hv in range(2):
                        wt = wp.tile([128, D // 128, FF // 2], bf16, tag="ws")
                        for kk in range(0, D // 128, 2):
                            nc.sync.dma_start(
                                wt[:, kk:kk + 2, :],
                                wup_v[:, kk:kk + 2, hv * (FF // 2):(hv + 1) * (FF // 2)])
                        wup_h.append(wt)
                    bup_sb = sp.tile([128, FF // 128], f32, tag="bup")
                    nc.sync.dma_start(bup_sb[:], bup_d[li].ap().rearrange("(m p) -> p m", p=128))
                    uT = ap_.tile([128, FF // 128, T], bf16, tag="uT")
                    for m in range(FF // 128):
                        pu = ps1.tile([128, T], f32, tag="pW")
                        mh, mo = m // (FF // 256), (m % (FF // 256)) * 128
                        for k in range(D // 128):
                            nc.tensor.matmul(pu[:], wup_h[mh][:, k, mo:mo + 128],
                                             xnT[:, k, :], start=(k == 0),
                                             stop=(k == D // 128 - 1))
                        nc.scalar.activation(uT[:, m, :], pu[:], AF.Gelu_apprx_tanh,
                                             bias=bup_sb[:, m:m + 1])
                    wdn_v = wdn_d[li].ap().rearrange("(kt p) m -> p kt m", p=128)
                    wdn_h = []
                    for hv in range(2):
                        wt = wp.tile([128, FF // 128, D // 2], bf16, tag="ws")
                        for kk in range(0, FF // 128, 8):
                            nc.sync.dma_start(
                                wt[:, kk:kk + 8, :],
                                wdn_v[:, kk:kk + 8, hv * (D // 2):(hv + 1) * (D // 2)])
                        wdn_h.append(wt)
                    bdn_bc = ap_.tile([128, D], f32, tag="bo_bc")
                    nc.sync.dma_start(
                        bdn_bc[:],
                        bass.AP(tensor=bdn_d[li], offset=0, ap=[[0, 128], [1, D]]))
                    for j in range(TJ):
                        for n in range(2):
                            pd = ps1.tile([128, 512], f32, tag="pW")
                            for k in range(FF // 128):
                                nc.tensor.matmul(pd[:], uT[:, k, j * 128:(j + 1) * 128],
                                                 wdn_h[n][:, k, :],
                                                 start=(k == 0), stop=(k == FF // 128 - 1))
                            xs = x_sb[:, j, n * 512:(n + 1) * 512]
                            nc.vector.tensor_add(xs, xs, pd[:])
                            nc.vector.tensor_add(xs, xs, bdn_bc[:, n * 512:(n + 1) * 512])

            # ============================ heads ============================
            with (
                tc.tile_pool(name="h_w", bufs=2) as wp2,
                tc.tile_pool(name="h_acts", bufs=1) as hp,
                tc.tile_pool(name="h_sp", bufs=2) as sp2,
            ):
                layer_norm_normalize(sp2)

                def head_hidden(w1_d_, b1_d_, tag):
                    """silu(xn @ w1 + b1) -> [128, TD//128, T] bf16 (transposed)."""
                    w1_sb = hp.tile([128, D // 128, TD], bf16, tag="w1")
                    nc.sync.dma_start(
                        w1_sb[:], w1_d_.ap().rearrange("(kt p) m -> p kt m", p=128))
                    b1_sb = hp.tile([128, TD // 128], f32, tag="b1")
                    nc.sync.dma_start(b1_sb[:], b1_d_.ap().rearrange("(m p) -> p m", p=128))
                    yT = hp.tile([128, TD // 128, T], bf16, tag=tag)
                    for m in range(TD // 128):
                        py = ps1.tile([128, T], f32, tag="pW")
                        for k in range(D // 128):
                            nc.tensor.matmul(py[:], w1_sb[:, k, m * 128:(m + 1) * 128],
                                             xnT[:, k, :], start=(k == 0),
                                             stop=(k == D // 128 - 1))
                        nc.scalar.activation(yT[:, m, :], py[:], AF.Silu,
                                             bias=b1_sb[:, m:m + 1])
                    return yT

                # ins/sub hidden + all-gather (over all 8 cores = all tokens)
                yiT = head_hidden(w1i_d, b1i_d, "yiT")
                ysT = head_hidden(w1s_d, b1s_d, "ysT")
                hid_in_view = hid_in.ap().rearrange("(hh kt p) q -> hh p kt q",
                                                    hh=2, p=128)
                nc.sync.dma_start(hid_in_view[0], yiT[:])
                nc.sync.dma_start(hid_in_view[1], ysT[:])
                nc.gpsimd.collective_compute(
                    "AllGather", Alu.bypass, replica_groups=groups_all,
                    ins=[hid_in.ap().opt()], outs=[hid_all.ap().opt()])
                hid_view = hid_all.ap().rearrange(
                    "(r hh kt p) q -> r hh p kt q", r=N_CORES, hh=2, p=128)

                # rates head (token-parallel, tiny)
                yrT = head_hidden(w1r_d, b1r_d, "yrT")
                w2r_sb = sp2.tile([128, TD // 128, 3], bf16, tag="w2r")
                nc.sync.dma_start(
                    w2r_sb[:], w2r_d.ap().rearrange("(kt p) n -> p kt n", p=128))
                b2r_bc = sp2.tile([128, 3], f32, tag="b2r")
                nc.sync.dma_start(
                    b2r_bc[:],
                    bass.AP(tensor=b2r_d, offset=0, ap=[[0, 128], [1, 3]]))
                pad_own_sb = sp2.tile([128, TJ], f32, tag="pad_own")
                nc.sync.dma_start(pad_own_sb[:],
                                  pad_own_d.ap().rearrange("(j p) -> p j", p=128))
                for j in range(TJ):
                    pr = ps1.tile([128, 3], f32, tag="pW")
                    for k in range(TD // 128):
                        nc.tensor.matmul(pr[:], yrT[:, k, j * 128:(j + 1) * 128],
                                         w2r_sb[:, k, :], start=(k == 0),
                                         stop=(k == TD // 128 - 1))
                    nc.vector.tensor_add(pr[:], pr[:], b2r_bc[:])
                    rt = sp2.tile([128, 3], f32, tag="rt")
                    # softplus(x) = ln(1 + exp(x)); pre-acts are O(1) so this
                    # is numerically fine
                    nc.scalar.activation(rt[:], pr[:], AF.Exp)
                    nc.vector.tensor_scalar_add(rt[:], rt[:], 1.0)
                    nc.scalar.activation(rt[:], rt[:], AF.Ln)
                    nc.vector.tensor_scalar_mul(rt[:], rt[:], pad_own_sb[:, j:j + 1])
                    nc.sync.dma_start(out_rates_d.ap()[j * 128:(j + 1) * 128, :], rt[:])

                pad_all_sb = hp.tile([128, 16], f32, tag="pad_all")
                nc.sync.dma_start(pad_all_sb[:],
                                  pad_all_d.ap().rearrange("(s p) -> p s", p=128))

                # big vocab-sharded head GEMM + streaming softmax
                # two token-waves of 1024 tokens each (8 row-blocks), so the
                # bf16 exp store only needs 8 slots of SBUF
                exp_stores = []
                for w in range(2):
                    est = hp.tile([128, 4, VS], bf16, tag=f"exp_store{w}")
                    exp_stores.append(est)
                for hi, (w2_d_, b2_d_, out_d_) in enumerate([
                    (w2i_d, b2i_d, out_ins_d),
                    (w2s_d, b2s_d, out_sub_d),
                ]):
                    w2_sb = wp2.tile([128, TD // 128, VS], bf16, tag="w2")
                    w2_v = w2_d_.ap().rearrange("(kt p) n -> p kt n", p=128)
                    for kk in range(TD // 128):
                        nc.sync.dma_start(w2_sb[:, kk:kk + 1, :], w2_v[:, kk:kk + 1, :])
                    if general_bias:
                        eb2_bc = hp.tile([128, VS], bf16, tag="b2")
                        nc.sync.dma_start(
                            eb2_bc[:],
                            bass.AP(tensor=b2_d_, offset=0, ap=[[0, 128], [1, VS]]))
                    for wave in range(4):
                        exp_store = exp_stores[wave % 2]
                        ar_i, ar_o = ar_in[hi * 4 + wave], ar_out[hi * 4 + wave]
                        sums = hp.tile([128, 4], f32, tag="sums")
                        for rr in range(2):
                            r = wave * 2 + rr
                            lhs = sp2.tile([128, TD // 128, T], bf16, tag="hlhs")
                            nc.sync.dma_start(lhs[:], hid_view[r, hi])
                            for j in range(TJ):
                                s = rr * TJ + j
                                sums_nt = sp2.tile([128, NVT], f32, tag="sums_nt")
                                for n in range(NVT):
                                    ph = ps1.tile([128, VT], f32, tag="pW")
                                    for k in range(TD // 128):
                                        nc.tensor.matmul(
                                            ph[:], lhs[:, k, j * 128:(j + 1) * 128],
                                            w2_sb[:, k, n * VT:(n + 1) * VT],
                                            start=(k == 0), stop=(k == TD // 128 - 1))
                                    es = exp_store[:, s, n * VT:(n + 1) * VT]
                                    if general_bias:
                                        er = sp2.tile([128, VT], bf16, tag="h_expr")
                                        nc.scalar.activation(er[:], ph[:], AF.Exp)
                                        # exp(l+b2) = exp(l)*exp(b2)
                                        nc.gpsimd.tensor_mul(
                                            es, er[:], eb2_bc[:, n * VT:(n + 1) * VT])
                                    else:
                                        nc.scalar.activation(es, ph[:], AF.Exp)
                                    nc.vector.reduce_sum(
                                        out=sums_nt[:, n:n + 1], in_=es,
                                        axis=mybir.AxisListType.X)
                                nc.vector.reduce_sum(out=sums[:, s:s + 1], in_=sums_nt[:],
                                                     axis=mybir.AxisListType.X)
                        # all-reduce softmax denominators over the vocab shards
                        nc.sync.dma_start(ar_i.ap().rearrange("(s p) -> p s", p=128),
                                          sums[:])
                        nc.gpsimd.collective_compute(
                            "AllReduce", Alu.add, replica_groups=groups_all,
                            ins=[ar_i.ap().opt()], outs=[ar_o.ap().opt()])
                        scale = hp.tile([128, 4], f32, tag="scale")
                        nc.sync.dma_start(scale[:],
                                          ar_o.ap().rearrange("(s p) -> p s", p=128))
                        nc.vector.reciprocal(scale[:], scale[:])
                        nc.vector.tensor_mul(scale[:], scale[:],
                                             pad_all_sb[:, wave * 4:(wave + 1) * 4])
                        for s in range(4):
                            row0 = (wave * 4 + s) * 128
                            for qr in range(4):
                                of = sp2.tile([128, VS // 4], f32, tag="of")
                                nc.vector.tensor_scalar_mul(
                                    of[:],
                                    exp_store[:, s, qr * (VS // 4):(qr + 1) * (VS // 4)],
                                    scale[:, s:s + 1])
                                nc.sync.dma_start(
                                    out_d_.ap()[row0:row0 + 128,
                                                qr * (VS // 4):(qr + 1) * (VS // 4)],
                                    of[:])

    nc.compile()
    return nc


def _get_nc(general_bias=False):
    key = ("nc", bool(general_bias))
    if key not in _cache:
        _cache[key] = _build(general_bias)
    return _cache[key]


def _time_emb(t):
    half = TD // 2
    freqs = np.exp(-math.log(10000.0) * np.arange(half, dtype=np.float64) / half)
    ang = t.astype(np.float64)[:, None] * freqs[None, :]
    return np.concatenate([np.sin(ang), np.cos(ang)], axis=-1).astype(np.float32)


def _prep_inputs(tokens, t, pad_mask, rand_mask, params):
    p = {k: np.asarray(v) for k, v in params.items()}
    tokens = np.asarray(tokens)
    t = np.asarray(t, dtype=np.float32)
    pad_mask = np.asarray(pad_mask)
    rand_mask = np.asarray(rand_mask)

    x0 = p["embed"][tokens.reshape(-1)].astype(np.float32) \
        + np.tile(p["pos"].astype(np.float32), (B, 1))

    causal = np.tril(np.ones((L, L), dtype=bool))
    mask = causal | rand_mask
    maskT = np.ascontiguousarray(mask.T.astype(BF16))

    pad_f = (~pad_mask).reshape(-1).astype(np.float32)

    te = _time_emb(t)  # [B, TD]

    def bf(x):
        return np.ascontiguousarray(x.astype(BF16))

    base = {}
    for i in range(NL):
        wqkv = p["wqkv"][i] * p["ln1_g"][i][:, None]
        bqkv = p["bqkv"][i] + p["ln1_b"][i] @ p["wqkv"][i]
        wqkv = wqkv.copy()
        bqkv = bqkv.copy()
        scale = 1.0 / math.sqrt(DH)
        wqkv[:, :D] *= scale
        bqkv[:D] *= scale
        # device-side column order is [K | V | Q]
        base[f"wqkv{i}"] = bf(np.concatenate(
            [wqkv[:, D:2 * D], wqkv[:, 2 * D:], wqkv[:, :D]], axis=1))
        base[f"bq{i}"] = bqkv[:D].astype(np.float32)
        base[f"bkv{i}"] = np.concatenate([bqkv[D:2 * D], bqkv[2 * D:]]).astype(np.float32)
        base[f"wo{i}"] = bf(p["wo"][i].reshape(H, DH, D).transpose(1, 0, 2))
        base[f"bo{i}"] = p["bo"][i].astype(np.float32)
        base[f"wup{i}"] = bf(p["w_up"][i] * p["ln2_g"][i][:, None])
        base[f"bup{i}"] = (p["b_up"][i] + p["ln2_b"][i] @ p["w_up"][i]).astype(np.float32)
        base[f"wdn{i}"] = bf(p["w_down"][i])
        base[f"bdn{i}"] = p["b_down"][i].astype(np.float32)

    heads = {}
    for nm, w1k, b1k, w2k, b2k in [
        ("r", "rate_w1", "rate_b1", "rate_w2", "rate_b2"),
        ("i", "ins_w1", "ins_b1", "ins_w2", "ins_b2"),
        ("s", "sub_w1", "sub_b1", "sub_w2", "sub_b2"),
    ]:
        w1 = p[w1k]
        heads[f"w1{nm}"] = bf(w1[:D] * p["lnf_g"][:, None])
        # per-batch effective b1: b1 + lnf_b @ w1[:D] + te[b] @ w1[D:]
        b1_base = p[b1k] + p["lnf_b"] @ w1[:D]
        heads[f"b1{nm}"] = (b1_base[None, :] + te @ w1[D:]).astype(np.float32)  # [B, TD]
        heads[f"w2{nm}"] = p[w2k]
        if nm == "r":
            heads[f"b2{nm}"] = p[b2k].astype(np.float32).reshape(1, -1)
        else:
            heads[f"b2{nm}"] = np.exp(p[b2k].astype(np.float64)).astype(BF16).reshape(1, -1)

    in_maps = []
    for core in range(N_CORES):
        b = core // 4
        t0 = core * T
        m = dict(base)
        m["x0"] = np.ascontiguousarray(x0[t0:t0 + T])
        m["maskT"] = np.ascontiguousarray(maskT[:, (core % 4) * T:(core % 4) * T + T])
        m["pad_all"] = pad_f
        m["pad_own"] = np.ascontiguousarray(pad_f[t0:t0 + T])
        for nm in ("r", "i", "s"):
            m[f"w1{nm}"] = heads[f"w1{nm}"]
            m[f"b1{nm}"] = np.ascontiguousarray(heads[f"b1{nm}"][b])
        m["w2r"] = bf(heads["w2r"])
        m["b2r"] = heads["b2r"]
        for nm in ("i", "s"):
            m[f"w2{nm}"] = bf(heads[f"w2{nm}"][:, core * VS:(core + 1) * VS])
            m[f"b2{nm}"] = np.ascontiguousarray(heads[f"b2{nm}"][:, core * VS:(core + 1) * VS])
        in_maps.append(m)
    return in_maps


def run_on_device(in_maps, trace=False, general_bias=False):
    from concourse.bass_utils import run_bass_kernel_spmd
    nc = _get_nc(general_bias)
    return run_bass_kernel_spmd(nc, in_maps, core_ids=list(range(N_CORES)),
                                trace=trace)


def kernel(tokens, t, pad_mask, rand_mask, params):
    in_maps = _prep_inputs(tokens, t, pad_mask, rand_mask, params)
    gb = bool(np.any(np.asarray(params["ins_b2"])) or
              np.any(np.asarray(params["sub_b2"])))
    res = run_on_device(in_maps, trace=False, general_bias=gb)
    outs = res.results
    ins = np.concatenate([outs[c]["out_ins"] for c in range(N_CORES)], axis=1)
    sub = np.concatenate([outs[c]["out_sub"] for c in range(N_CORES)], axis=1)
    rates = np.concatenate([outs[c]["out_rates"] for c in range(N_CORES)], axis=0)
    return (rates.reshape(B, L, 3).astype(np.float32),
            ins.reshape(B, L, V).astype(np.float32),
            sub.reshape(B, L, V).astype(np.float32))


# revision 37
# speedup vs baseline: 1.0434x; 1.0434x over previous
"""Trainium2 Bass kernel for nn_AdaptedEditFlowsTransformer.

Self-contained: takes full (unsharded) inputs, returns the full output tuple
(rates, ins, sub) matching the reference.

Sharding over 8 NeuronCores:
  - transformer trunk: sequence-parallel, 256 tokens per core (4 cores per
    batch element); K/V all-gathered per layer within each batch's core group.
  - output heads (ins/sub, V=32000): vocab-parallel, 4000 columns per core;
    softmax denominators combined with one small AllReduce per head.
  - rates head: token-parallel (each core emits its 256 tokens).

Host-side prep is limited to layout/dtype work: embedding row gather, bf16
weight casts, folding LayerNorm gains/biases + 1/sqrt(dh) into adjacent
matmul weights, and the time-embedding contribution folded into head biases.
"""

import sys

sys.path.insert(0, "/opt/trn_rl_repo")

import math

import numpy as np
import ml_dtypes

BF16 = ml_dtypes.bfloat16

B, L, D, H, NL = 2, 1024, 1024, 16, 2
V, TD, FF = 32000, 512, 4096
DH = D // H  # 64
N_CORES = 8
T = (B * L) // N_CORES  # 256 tokens per core
TJ = T // 128  # 2 token sub-tiles per core
VS = V // N_CORES  # 4000 vocab cols per core
NVT = 8  # vocab n-tiles per core
VT = VS // NVT  # 500
MASK_NEG = -60.0
EPS = 1e-5

_cache = {}


def _build(general_bias):
    import concourse.bass as bass
    import concourse.tile as tile
    import concourse.mybir as mybir
    from concourse import bacc
    from concourse.masks import make_identity

    f32 = mybir.dt.float32
    bf16 = mybir.dt.bfloat16
    AF = mybir.ActivationFunctionType
    Alu = mybir.AluOpType

    nc = bacc.Bacc("TRN2", target_bir_lowering=False, debug=False,
                   num_devices=N_CORES)

    # ---------------- DRAM parameters ----------------
    def din(name, shape, dt=f32):
        return nc.dram_tensor(name, shape, dt, kind="ExternalInput")

    x0_d = din("x0", [T, D])                     # embed[tokens]+pos, own tokens
    maskT_d = din("maskT", [L, T], bf16)         # attn 0/1 multiplier, [k, own q]
    pad_all_d = din("pad_all", [B * L])          # (~pad) as f32, all tokens
    pad_own_d = din("pad_own", [T])              # (~pad) f32, own tokens
    wqkv_d, bq_d, bkv_d, wo_d, bo_d = [], [], [], [], []
    wup_d, bup_d, wdn_d, bdn_d = [], [], [], []
    for i in range(NL):
        wqkv_d.append(din(f"wqkv{i}", [D, 3 * D], bf16))
        bq_d.append(din(f"bq{i}", [D]))
        bkv_d.append(din(f"bkv{i}", [2 * D]))
        wo_d.append(din(f"wo{i}", [DH, H, D], bf16))   # host pre-shuffled
        bo_d.append(din(f"bo{i}", [D]))
        wup_d.append(din(f"wup{i}", [D, FF], bf16))
        bup_d.append(din(f"bup{i}", [FF]))
        wdn_d.append(din(f"wdn{i}", [FF, D], bf16))
        bdn_d.append(din(f"bdn{i}", [D]))
    w1r_d = din("w1r", [D, TD], bf16)
    b1r_d = din("b1r", [TD])
    w2r_d = din("w2r", [TD, 3], bf16)
    b2r_d = din("b2r", [1, 3])
    w1i_d = din("w1i", [D, TD], bf16)
    b1i_d = din("b1i", [TD])
    w2i_d = din("w2i", [TD, VS], bf16)
    b2i_d = din("b2i", [1, VS], bf16)
    w1s_d = din("w1s", [D, TD], bf16)
    b1s_d = din("b1s", [TD])
    w2s_d = din("w2s", [TD, VS], bf16)
    b2s_d = din("b2s", [1, VS], bf16)

    out_ins_d = nc.dram_tensor("out_ins", [B * L, VS], f32, kind="ExternalOutput")
    out_sub_d = nc.dram_tensor("out_sub", [B * L, VS], f32, kind="ExternalOutput")
    out_rates_d = nc.dram_tensor("out_rates", [T, 3], f32, kind="ExternalOutput")

    # collective bounce buffers
    kv_in = [nc.dram_tensor(f"kv_in{i}", [2 * D * T], bf16) for i in range(NL)]
    kv_all = [nc.dram_tensor(f"kv_all{i}", [4 * 2 * D * T], bf16)
              for i in range(NL)]
    hid_in = nc.dram_tensor("hid_in", [2 * TD, T], bf16)
    hid_all = nc.dram_tensor("hid_all", [N_CORES * 2 * TD, T], bf16,
                             addr_space="Shared")
    ar_in = [nc.dram_tensor(f"ar_in{k}", [B * L // 4], f32) for k in range(8)]
    ssum_d = nc.dram_tensor("ssum_d", [H, T], f32)
    ar_out = [nc.dram_tensor(f"ar_out{k}", [B * L // 4], f32, addr_space="Shared")
              for k in range(8)]

    groups_batch = [[0, 1, 2, 3], [4, 5, 6, 7]]
    groups_all = [list(range(N_CORES))]

    with tile.TileContext(nc) as tc:
        with (
            tc.tile_pool(name="persist", bufs=1) as pp,
            tc.tile_pool(name="ps1", bufs=4, space="PSUM") as ps1,
            tc.tile_pool(name="ps2", bufs=2, space="PSUM") as ps2,
        ):
            ident = pp.tile([128, 128], f32, tag="ident")
            make_identity(nc, ident[:])
            ident_bf = pp.tile([64, 64], bf16, tag="ident_bf")
            make_identity(nc, ident_bf[:])
            ones65 = pp.tile([65, 128], f32, tag="ones65")
            nc.vector.memset(ones65[:], 1.0)
            eps_t = pp.tile([128, 1], f32, tag="eps")
            nc.vector.memset(eps_t[:], EPS)

            # resident activations
            x_sb = pp.tile([128, TJ, D], f32, tag="x")         # residual
            x0_v = x0_d.ap().rearrange("(j p) d -> p j d", p=128)
            for j in range(TJ):
                nc.sync.dma_start(x_sb[:, j:j + 1, :], x0_v[:, j:j + 1, :])
            xnT = pp.tile([128, D // 128, T], bf16, tag="xnT")  # normed, transposed

            def layer_norm_normalize(sp):
                """x_sb -> xnT (pure (x-m)*rstd, transposed, bf16)."""
                for j in range(TJ):
                    xj = x_sb[:, j, :]
                    stats = sp.tile([128, D // 512, 6], f32, tag="ln_stats")
                    for g in range(D // 512):
                        nc.vector.bn_stats(stats[:, g, :], xj[:, g * 512:(g + 1) * 512])
                    mv = sp.tile([128, 2], f32, tag="ln_mv")
                    nc.vector.bn_aggr(mv[:], stats[:])
                    rstd = sp.tile([128, 1], f32, tag="ln_rstd")
                    nc.scalar.activation(rstd[:], mv[:, 1:2], AF.Sqrt, bias=eps_t[:])
                    nc.vector.reciprocal(rstd[:], rstd[:])
                    nmr = sp.tile([128, 1], f32, tag="ln_nmr")
                    nc.vector.tensor_scalar(nmr[:], mv[:, 0:1], rstd[:], -1.0,
                                            Alu.mult, Alu.mult)
                    xn = sp.tile([128, D], f32, tag="ln_xn")
                    nc.vector.tensor_scalar(xn[:], xj, rstd[:], nmr[:],
                                            Alu.mult, Alu.add)
                    for dt_ in range(D // 128):
                        pt = ps2.tile([128, 128], f32, tag="tp")
                        nc.tensor.transpose(pt[:], xn[:, dt_ * 128:(dt_ + 1) * 128],
                                            ident[:])
                        nc.vector.tensor_copy(xnT[:, dt_, j * 128:(j + 1) * 128], pt[:])

            # ============================ trunk ============================
            with (
                tc.tile_pool(name="t_w", bufs=3) as wp,
                tc.tile_pool(name="t_acts", bufs=1) as ap_,
                tc.tile_pool(name="t_sp", bufs=3) as sp,
            ):
                maskT_sb = ap_.tile([128, L // 128, T], bf16, tag="maskT")
                nc.sync.dma_start(
                    maskT_sb[:], maskT_d.ap().rearrange("(kt p) q -> p kt q", p=128))
                for li in range(NL):
                    layer_norm_normalize(sp)

                    # ---- qkv projection ----
                    wqkv_v = wqkv_d[li].ap().rearrange("(kt p) m -> p kt m", p=128)
                    wqkv_h = []
                    for hv in range(2):
                        wt = wp.tile([128, D // 128, 3 * D // 2], bf16, tag="ws")
                        for kk in range(0, D // 128, 2):
                            nc.sync.dma_start(
                                wt[:, kk:kk + 2, :],
                                wqkv_v[:, kk:kk + 2,
                                       hv * (3 * D // 2):(hv + 1) * (3 * D // 2)])
                        wqkv_h.append(wt)
                    def wqkv_sl(k, lo, hi):
                        half = lo // (3 * D // 2)
                        off = lo - half * (3 * D // 2)
                        return wqkv_h[half][:, k, off:off + (hi - lo)]
                    bq_sb = sp.tile([128, 8], f32, tag="bq")
                    nc.sync.dma_start(bq_sb[:], bq_d[li].ap().rearrange("(m p) -> p m", p=128))
                    bk_sb = sp.tile([128, 8], f32, tag="bkv")
                    nc.sync.dma_start(bk_sb[:],
                                      bkv_d[li].ap()[0:D].rearrange("(m p) -> p m", p=128))
                    bv_bc = ap_.tile([128, D], f32, tag="bv_bc")
                    nc.sync.dma_start(
                        bv_bc[:],
                        bass.AP(tensor=bkv_d[li], offset=D, ap=[[0, 128], [1, D]]))

                    kvT = ap_.tile([128, 8, T], bf16, tag="kvoT")   # K, [dim, tok]
                    v_nat_own = ap_.tile([128, TJ, D], bf16, tag="vno")
                    for m in range(8):  # K tiles first so the all-gather launches early
                        pkv = ps1.tile([128, T], f32, tag="pW")
                        for k in range(D // 128):
                            nc.tensor.matmul(pkv[:], wqkv_sl(k, m * 128, (m + 1) * 128),
                                             xnT[:, k, :], start=(k == 0),
                                             stop=(k == D // 128 - 1))
                        nc.vector.tensor_scalar_add(kvT[:, m, :], pkv[:], bk_sb[:, m:m + 1])
                        nc.sync.dma_start(
                            bass.AP(tensor=kv_in[li], offset=m * 128 * T,
                                    ap=[[T, 128], [1, T]]),
                            kvT[:, m, :])
                    for j in range(TJ):  # V directly in natural [token, dim] layout
                        for n in range(2):
                            pv = ps1.tile([128, 512], f32, tag="pW")
                            for k in range(D // 128):
                                nc.tensor.matmul(
                                    pv[:], xnT[:, k, j * 128:(j + 1) * 128],
                                    wqkv_sl(k, D + n * 512, D + (n + 1) * 512),
                                    start=(k == 0), stop=(k == D // 128 - 1))
                            nc.vector.tensor_add(v_nat_own[:, j, n * 512:(n + 1) * 512],
                                                 pv[:], bv_bc[:, n * 512:(n + 1) * 512])
                            nc.sync.dma_start(
                                bass.AP(tensor=kv_in[li],
                                        offset=D * T + j * 128 * D + n * 512,
                                        ap=[[D, 128], [1, 512]]),
                                v_nat_own[:, j, n * 512:(n + 1) * 512])
                    nc.gpsimd.collective_compute(
                        "AllGather", Alu.bypass, replica_groups=groups_batch,
                        ins=[kv_in[li].ap().opt()], outs=[kv_all[li].ap().opt()])

                    qtmp = ap_.tile([128, 8, T], bf16, tag="qT")  # overlaps the AG
                    for m in range(8):
                        pq = ps1.tile([128, T], f32, tag="pW")
                        for k in range(D // 128):
                            nc.tensor.matmul(pq[:], wqkv_sl(k, 2 * D + m * 128, 2 * D + (m + 1) * 128),
                                             xnT[:, k, :], start=(k == 0),
                                             stop=(k == D // 128 - 1))
                        nc.vector.tensor_scalar_add(qtmp[:, m, :], pq[:], bq_sb[:, m:m + 1])

                    # ---- attention ----
                    wo_sb = wp.tile([64, H, D], bf16, tag="ws")
                    for kk in range(0, H, 8):
                        nc.sync.dma_start(wo_sb[:, kk:kk + 8, :],
                                          wo_d[li].ap()[:, kk:kk + 8, :])
                    oT = ap_.tile([64, H, T], bf16, tag="oT")
                    CH = 2 * D * T  # flat chunk stride in kv_all
                    for h in [hh for par in range(2) for hh in range(par, H, 2)]:
                        po = (h % 2) * 64
                        kT2 = sp.tile([128, 4, T], bf16, tag="kT")
                        nc.sync.dma_start(
                            kT2[po:po + 64, :, :],
                            bass.AP(tensor=kv_all[li],
                                    offset=(h // 2) * 128 * T + po * T,
                                    ap=[[T, 64], [CH, 4], [1, T]]))
                        # v for this head + ones row (softmax denominators)
                        v_aug = sp.tile([128, L // 128, DH + 1], bf16, tag="vnat")
                        nc.vector.memset(v_aug[:, :, DH:DH + 1], 1.0)
                        _va = v_aug[:]
                        for jj in range(TJ):
                            nc.sync.dma_start(
                                bass.AP(tensor=_va.tensor,
                                        offset=_va.offset + jj * (DH + 1),
                                        ap=[[8 * (DH + 1), 128],
                                            [2 * (DH + 1), 4], [1, DH]]),
                                bass.AP(tensor=kv_all[li],
                                        offset=D * T + h * DH + jj * 128 * D,
                                        ap=[[D, 128], [CH, 4], [1, DH]]))
                        expT = sp.tile([128, L // 128, T], bf16, tag="expT")
                        for kt in range(L // 128):
                            ps = ps1.tile([128, T], f32, tag="pW")
                            nc.tensor.matmul(
                                ps[:],
                                kT2[po:po + 64, kt // 2, (kt % 2) * 128:(kt % 2) * 128 + 128],
                                qtmp[po:po + 64, h // 2, :], start=True, stop=True)
                            er = sp.tile([128, T], bf16, tag="expr")
                            nc.scalar.activation(er[:], ps[:], AF.Exp)
                            nc.gpsimd.tensor_mul(expT[:, kt, :], er[:], maskT_sb[:, kt, :])
                        pav = ps2.tile([65, T], f32, tag="pav")
                        for kt in range(L // 128):
                            nc.tensor.matmul(pav[:], v_aug[:, kt, :], expT[:, kt, :],
                                             start=(kt == 0), stop=(kt == L // 128 - 1))
                        ssum = sp.tile([65, T], f32, tag="recip")
                        nc.vector.tensor_copy(ssum[64:65, :], pav[64:65, :])
                        nc.sync.dma_start(ssum_d.ap()[h, :], ssum[64:65, :])
                        rbs = sp.tile([64, T], f32, tag="rbs")
                        nc.sync.dma_start(
                            rbs[:], bass.AP(tensor=ssum_d, offset=h * T,
                                            ap=[[0, 64], [1, T]]))
                        rb = sp.tile([64, T], f32, tag="rb")
                        nc.vector.reciprocal(rb[:], rbs[:])
                        nc.vector.tensor_mul(oT[:, h, :], pav[0:64, :], rb[:])

                    # ---- o @ wo + bo, residual add ----
                    bo_bc = ap_.tile([128, D], f32, tag="bo_bc")
                    nc.sync.dma_start(
                        bo_bc[:],
                        bass.AP(tensor=bo_d[li], offset=0, ap=[[0, 128], [1, D]]))
                    for j in range(TJ):
                        for n in range(2):
                            pw = ps1.tile([128, 512], f32, tag="pW")
                            for h in range(H):
                                nc.tensor.matmul(pw[:], oT[:, h, j * 128:(j + 1) * 128],
                                                 wo_sb[:, h, n * 512:(n + 1) * 512],
                                                 start=(h == 0), stop=(h == H - 1))
                            xs = x_sb[:, j, n * 512:(n + 1) * 512]
                            nc.vector.tensor_add(xs, xs, pw[:])
                            nc.vector.tensor_add(xs, xs, bo_bc[:, n * 512:(n + 1) * 512])

                    # ---- MLP ----
                    layer_norm_normalize(sp)
                    wup_v = wup_d[li].ap().rearrange("(kt p) m -> p kt m", p=128)
                    wup_h = []
                    for hv in range(2):
                        wt = wp.tile([128, D // 128, FF // 2], bf16, tag="ws")
                        for kk in range(0, D // 128, 2):
                            nc.sync.dma_start(
                                wt[:, kk:kk + 2, :],
                                wup_v[:, kk:kk + 2, hv * (FF // 2):(hv + 1) * (FF // 2)])
                        wup_h.append(wt)
                    bup_sb = sp.tile([128, FF // 128], f32, tag="bup")
                    nc.sync.dma_start(bup_sb[:], bup_d[li].ap().rearrange("(m p) -> p m", p=128))
                    uT = ap_.tile([128, FF // 128, T], bf16, tag="uT")
                    for m in range(FF // 128):
                        pu = ps1.tile([128, T], f32, tag="pW")
                        mh, mo = m // (FF // 256), (m % (FF // 256)) * 128
                        for k in range(D // 128):
                            nc.tensor.matmul(pu[:], wup_h[mh][:, k, mo:mo + 128],
                                             xnT[:, k, :], start=(k == 0),
                                             stop=(k == D // 128 - 1))
                        nc.scalar.activation(uT[:, m, :], pu[:], AF.Gelu_apprx_tanh,
                                             bias=bup_sb[:, m:m + 1])
                    wdn_v = wdn_d[li].ap().rearrange("(kt p) m -> p kt m", p=128)
                    wdn_h = []
                    for hv in range(2):
                        wt = wp.tile([128, FF // 128, D // 2], bf16, tag="ws")
                        for kk in range(0, FF // 128, 8):
                            nc.sync.dma_start(
                                wt[:, kk:kk + 8, :],
                                wdn_v[:, kk:kk + 8, hv * (D // 2):(hv + 1) * (D // 2)])
                        wdn_h.append(wt)
                    bdn_bc = ap_.tile([128, D], f32, tag="bo_bc")
                    nc.sync.dma_start(
                        bdn_bc[:],
                        bass.AP(tensor=bdn_d[li], offset=0, ap=[[0, 128], [1, D]]))
                    for j in range(TJ):
                        for n in range(2):
                            pd = ps1.tile([128, 512], f32, tag="pW")
                            for k in range(FF // 128):
                                nc.tensor.matmul(pd[:], uT[:, k, j * 128:(j + 1) * 128],
                                                 wdn_h[n][:, k, :],
                                                 start=(k == 0), stop=(k == FF // 128 - 1))
                            xs = x_sb[:, j, n * 512:(n + 1) * 512]
                            nc.vector.tensor_add(xs, xs, pd[:])
                            nc.vector.tensor_add(xs, xs, bdn_bc[:, n * 512:(n + 1) * 512])

            # ============================ heads ============================
            with (
                tc.tile_pool(name="h_w", bufs=2) as wp2,
                tc.tile_pool(name="h_acts", bufs=1) as hp,
                tc.tile_pool(name="h_sp", bufs=2) as sp2,
            ):
                layer_norm_normalize(sp2)

                def head_hidden(w1_d_, b1_d_, tag):
                    """silu(xn @ w1 + b1) -> [128, TD//128, T] bf16 (transposed)."""
                    w1_sb = hp.tile([128, D // 128, TD], bf16, tag="w1")
                    nc.sync.dma_start(
                        w1_sb[:], w1_d_.ap().rearrange("(kt p) m -> p kt m", p=128))
                    b1_sb = hp.tile([128, TD // 128], f32, tag="b1")
                    nc.sync.dma_start(b1_sb[:], b1_d_.ap().rearrange("(m p) -> p m", p=128))
                    yT = hp.tile([128, TD // 128, T], bf16, tag=tag)
                    for m in range(TD // 128):
                        py = ps1.tile([128, T], f32, tag="pW")
                        for k in range(D // 128):
                            nc.tensor.matmul(py[:], w1_sb[:, k, m * 128:(m + 1) * 128],
                                             xnT[:, k, :], start=(k == 0),
                                             stop=(k == D // 128 - 1))
                        nc.scalar.activation(yT[:, m, :], py[:], AF.Silu,
                                             bias=b1_sb[:, m:m + 1])
                    return yT

                # ins/sub hidden + all-gather (over all 8 cores = all tokens)
                yiT = head_hidden(w1i_d, b1i_d, "yiT")
                ysT = head_hidden(w1s_d, b1s_d, "ysT")
                hid_in_view = hid_in.ap().rearrange("(hh kt p) q -> hh p kt q",
                                                    hh=2, p=128)
                nc.sync.dma_start(hid_in_view[0], yiT[:])
                nc.sync.dma_start(hid_in_view[1], ysT[:])
                nc.gpsimd.collective_compute(
                    "AllGather", Alu.bypass, replica_groups=groups_all,
                    ins=[hid_in.ap().opt()], outs=[hid_all.ap().opt()])
                hid_view = hid_all.ap().rearrange(
                    "(r hh kt p) q -> r hh p kt q", r=N_CORES, hh=2, p=128)

                pad_all_sb = hp.tile([128, 16], f32, tag="pad_all")
                nc.sync.dma_start(pad_all_sb[:],
                                  pad_all_d.ap().rearrange("(s p) -> p s", p=128))

                # big vocab-sharded head GEMM + streaming softmax
                # two token-waves of 1024 tokens each (8 row-blocks), so the
                # bf16 exp store only needs 8 slots of SBUF
                exp_stores = []
                for w in range(2):
                    est = hp.tile([128, 4, VS], bf16, tag=f"exp_store{w}")
                    exp_stores.append(est)
                for hi, (w2_d_, b2_d_, out_d_) in enumerate([
                    (w2i_d, b2i_d, out_ins_d),
                    (w2s_d, b2s_d, out_sub_d),
                ]):
                    w2_sb = wp2.tile([128, TD // 128, VS], bf16, tag="w2")
                    w2_v = w2_d_.ap().rearrange("(kt p) n -> p kt n", p=128)
                    for kk in range(TD // 128):
                        nc.sync.dma_start(w2_sb[:, kk:kk + 1, :], w2_v[:, kk:kk + 1, :])
                    if general_bias:
                        eb2_bc = hp.tile([128, VS], bf16, tag="b2")
                        nc.sync.dma_start(
                            eb2_bc[:],
                            bass.AP(tensor=b2_d_, offset=0, ap=[[0, 128], [1, VS]]))
                    for wave in range(4):
                        exp_store = exp_stores[wave % 2]
                        ar_i, ar_o = ar_in[hi * 4 + wave], ar_out[hi * 4 + wave]
                        sums = hp.tile([128, 4], f32, tag="sums")
                        for rr in range(2):
                            r = wave * 2 + rr
                            lhs = sp2.tile([128, TD // 128, T], bf16, tag="hlhs")
                            nc.sync.dma_start(lhs[:], hid_view[r, hi])
                            for j in range(TJ):
                                s = rr * TJ + j
                                sums_nt = sp2.tile([128, NVT], f32, tag="sums_nt")
                                for n in range(NVT):
                                    ph = ps1.tile([128, VT], f32, tag="pW")
                                    for k in range(TD // 128):
                                        nc.tensor.matmul(
                                            ph[:], lhs[:, k, j * 128:(j + 1) * 128],
                                            w2_sb[:, k, n * VT:(n + 1) * VT],
                                            start=(k == 0), stop=(k == TD // 128 - 1))
                                    es = exp_store[:, s, n * VT:(n + 1) * VT]
                                    if general_bias:
                                        er = sp2.tile([128, VT], bf16, tag="h_expr")
                                        nc.scalar.activation(er[:], ph[:], AF.Exp)
                                        # exp(l+b2) = exp(l)*exp(b2)
                                        nc.gpsimd.tensor_mul(
                                            es, er[:], eb2_bc[:, n * VT:(n + 1) * VT])
                                    else:
                                        nc.scalar.activation(es, ph[:], AF.Exp)
                                    nc.vector.reduce_sum(
                                        out=sums_nt[:, n:n + 1], in_=es,
                                        axis=mybir.AxisListType.X)
                                nc.vector.reduce_sum(out=sums[:, s:s + 1], in_=sums_nt[:],
                                                     axis=mybir.AxisListType.X)
                        # all-reduce softmax denominators over the vocab shards
                        nc.sync.dma_start(ar_i.ap().rearrange("(s p) -> p s", p=128),
                                          sums[:])
                        nc.gpsimd.collective_compute(
                            "AllReduce", Alu.add, replica_groups=groups_all,
                            ins=[ar_i.ap().opt()], outs=[ar_o.ap().opt()])
                        scale = hp.tile([128, 4], f32, tag="scale")
                        nc.sync.dma_start(scale[:],
                                          ar_o.ap().rearrange("(s p) -> p s", p=128))
                        nc.vector.reciprocal(scale[:], scale[:])
                        nc.vector.tensor_mul(scale[:], scale[:],
                                             pad_all_sb[:, wave * 4:(wave + 1) * 4])
                        for s in range(4):
                            row0 = (wave * 4 + s) * 128
                            for qr in range(4):
                                of = sp2.tile([128, VS // 4], f32, tag="of")
                                nc.vector.tensor_scalar_mul(
                                    of[:],
                                    exp_store[:, s, qr * (VS // 4):(qr + 1) * (VS // 4)],
                                    scale[:, s:s + 1])
                                nc.sync.dma_start(
                                    out_d_.ap()[row0:row0 + 128,
                                                qr * (VS // 4):(qr + 1) * (VS // 4)],
                                    of[:])

                # rates head (token-parallel, tiny)
                yrT = head_hidden(w1r_d, b1r_d, "yrT")
                w2r_sb = sp2.tile([128, TD // 128, 3], bf16, tag="w2r")
                nc.sync.dma_start(
                    w2r_sb[:], w2r_d.ap().rearrange("(kt p) n -> p kt n", p=128))
                b2r_bc = sp2.tile([128, 3], f32, tag="b2r")
                nc.sync.dma_start(
                    b2r_bc[:],
                    bass.AP(tensor=b2r_d, offset=0, ap=[[0, 128], [1, 3]]))
                pad_own_sb = sp2.tile([128, TJ], f32, tag="pad_own")
                nc.sync.dma_start(pad_own_sb[:],
                                  pad_own_d.ap().rearrange("(j p) -> p j", p=128))
                for j in range(TJ):
                    pr = ps1.tile([128, 3], f32, tag="pW")
                    for k in range(TD // 128):
                        nc.tensor.matmul(pr[:], yrT[:, k, j * 128:(j + 1) * 128],
                                         w2r_sb[:, k, :], start=(k == 0),
                                         stop=(k == TD // 128 - 1))
                    nc.vector.tensor_add(pr[:], pr[:], b2r_bc[:])
                    rt = sp2.tile([128, 3], f32, tag="rt")
                    # softplus(x) = ln(1 + exp(x)); pre-acts are O(1) so this
                    # is numerically fine
                    nc.scalar.activation(rt[:], pr[:], AF.Exp)
                    nc.vector.tensor_scalar_add(rt[:], rt[:], 1.0)
                    nc.scalar.activation(rt[:], rt[:], AF.Ln)
                    nc.vector.tensor_scalar_mul(rt[:], rt[:], pad_own_sb[:, j:j + 1])
                    nc.sync.dma_start(out_rates_d.ap()[j * 128:(j + 1) * 128, :], rt[:])

    nc.compile()
    return nc


def _get_nc(general_bias=False):
    key = ("nc", bool(general_bias))
    if key not in _cache:
        _cache[key] = _build(general_bias)
    return _cache[key]


def _time_emb(t):
    half = TD // 2
    freqs = np.exp(-math.log(10000.0) * np.arange(half, dtype=np.float64) / half)
    ang = t.astype(np.float64)[:, None] * freqs[None, :]
    return np.concatenate([np.sin(ang), np.cos(ang)], axis=-1).astype(np.float32)


def _prep_inputs(tokens, t, pad_mask, rand_mask, params):
    p = {k: np.asarray(v) for k, v in params.items()}
    tokens = np.asarray(tokens)
    t = np.asarray(t, dtype=np.float32)
    pad_mask = np.asarray(pad_mask)
    rand_mask = np.asarray(rand_mask)

    x0 = p["embed"][tokens.reshape(-1)].astype(np.float32) \
        + np.tile(p["pos"].astype(np.float32), (B, 1))

    causal = np.tril(np.ones((L, L), dtype=bool))
    mask = causal | rand_mask
    maskT = np.ascontiguousarray(mask.T.astype(BF16))

    pad_f = (~pad_mask).reshape(-1).astype(np.float32)

    te = _time_emb(t)  # [B, TD]

    def bf(x):
        return np.ascontiguousarray(x.astype(BF16))

    base = {}
    for i in range(NL):
        wqkv = p["wqkv"][i] * p["ln1_g"][i][:, None]
        bqkv = p["bqkv"][i] + p["ln1_b"][i] @ p["wqkv"][i]
        wqkv = wqkv.copy()
        bqkv = bqkv.copy()
        scale = 1.0 / math.sqrt(DH)
        wqkv[:, :D] *= scale
        bqkv[:D] *= scale
        # device-side column order is [K | V | Q]
        base[f"wqkv{i}"] = bf(np.concatenate(
            [wqkv[:, D:2 * D], wqkv[:, 2 * D:], wqkv[:, :D]], axis=1))
        base[f"bq{i}"] = bqkv[:D].astype(np.float32)
        base[f"bkv{i}"] = np.concatenate([bqkv[D:2 * D], bqkv[2 * D:]]).astype(np.float32)
        base[f"wo{i}"] = bf(p["wo"][i].reshape(H, DH, D).transpose(1, 0, 2))
        base[f"bo{i}"] = p["bo"][i].astype(np.float32)
        base[f"wup{i}"] = bf(p["w_up"][i] * p["ln2_g"][i][:, None])
        base[f"bup{i}"] = (p["b_up"][i] + p["ln2_b"][i] @ p["w_up"][i]).astype(np.float32)
        base[f"wdn{i}"] = bf(p["w_down"][i])
        base[f"bdn{i}"] = p["b_down"][i].astype(np.float32)

    heads = {}
    for nm, w1k, b1k, w2k, b2k in [
        ("r", "rate_w1", "rate_b1", "rate_w2", "rate_b2"),
        ("i", "ins_w1", "ins_b1", "ins_w2", "ins_b2"),
        ("s", "sub_w1", "sub_b1", "sub_w2", "sub_b2"),
    ]:
        w1 = p[w1k]
        heads[f"w1{nm}"] = bf(w1[:D] * p["lnf_g"][:, None])
        # per-batch effective b1: b1 + lnf_b @ w1[:D] + te[b] @ w1[D:]
        b1_base = p[b1k] + p["lnf_b"] @ w1[:D]
        heads[f"b1{nm}"] = (b1_base[None, :] + te @ w1[D:]).astype(np.float32)  # [B, TD]
        heads[f"w2{nm}"] = p[w2k]
        if nm == "r":
            heads[f"b2{nm}"] = p[b2k].astype(np.float32).reshape(1, -1)
        else:
            heads[f"b2{nm}"] = np.exp(p[b2k].astype(np.float64)).astype(BF16).reshape(1, -1)

    in_maps = []
    for core in range(N_CORES):
        b = core // 4
        t0 = core * T
        m = dict(base)
        m["x0"] = np.ascontiguousarray(x0[t0:t0 + T])
        m["maskT"] = np.ascontiguousarray(maskT[:, (core % 4) * T:(core % 4) * T + T])
        m["pad_all"] = pad_f
        m["pad_own"] = np.ascontiguousarray(pad_f[t0:t0 + T])
        for nm in ("r", "i", "s"):
            m[f"w1{nm}"] = heads[f"w1{nm}"]
            m[f"b1{nm}"] = np.ascontiguousarray(heads[f"b1{nm}"][b])
        m["w2r"] = bf(heads["w2r"])
        m["b2r"] = heads["b2r"]
        for nm in ("i", "s"):
            m[f"w2{nm}"] = bf(heads[f"w2{nm}"][:, core * VS:(core + 1) * VS])
            m[f"b2{nm}"] = np.ascontiguousarray(heads[f"b2{nm}"][:, core * VS:(core + 1) * VS])
        in_maps.append(m)
    return in_maps


def run_on_device(in_maps, trace=False, general_bias=False):
    from concourse.bass_utils import run_bass_kernel_spmd
    nc = _get_nc(general_bias)
    return run_bass_kernel_spmd(nc, in_maps, core_ids=list(range(N_CORES)),
                                trace=trace)


def kernel(tokens, t, pad_mask, rand_mask, params):
    in_maps = _prep_inputs(tokens, t, pad_mask, rand_mask, params)
    gb = bool(np.any(np.asarray(params["ins_b2"])) or
              np.any(np.asarray(params["sub_b2"])))
    res = run_on_device(in_maps, trace=False, general_bias=gb)
    outs = res.results
    ins = np.concatenate([outs[c]["out_ins"] for c in range(N_CORES)], axis=1)
    sub = np.concatenate([outs[c]["out_sub"] for c in range(N_CORES)], axis=1)
    rates = np.concatenate([outs[c]["out_rates"] for c in range(N_CORES)], axis=0)
    return (rates.reshape(B, L, 3).astype(np.float32),
            ins.reshape(B, L, V).astype(np.float32),
            sub.reshape(B, L, V).astype(np.float32))
